# revision 1
# baseline (speedup 1.0000x reference)
"""Trainium2 Bass kernel for nn_Degrade: depthwise 13x13 blur + 4x downsample.

Reference computation (per sample, per channel):
  replicate-pad by 6, 13x13 cross-correlation with the per-sample kernel,
  stride-4 downsample: im [8,4,1024,1024] f32, kernel [8,1,13,13] f32
  -> out [8,4,256,256] f32.

Sharding: pure data parallel, one sample per NeuronCore (8 cores).

Per-core algorithm (single matmul pass, contraction over image rows):
  out[oy, ox] = sum_kx sum_y  Wb_kx[y, oy] * Impad[y, 4*ox + kx]
where Wb_kx[y, oy] = kernel[y - 4*oy, kx] is a banded matrix built on host.
Host prep (numpy):
  - replicate-pad image to [4, 1036, 1036]
  - polyphase-split x (x % 4) so every kx tap reads a CONTIGUOUS window of a
    phase plane (TensorE streams strided rhs at ~half rate, contiguous at
    1 col/cycle)
  - rows regrouped into 9 row-block tiles on two overlapping 128-row grids
    (oy tiles of {124,124,8}) so every matmul's rhs starts at partition 0
  - weights duplicated per row-block index j ([y, j, kx, 128]) so each j group
    is one contiguous DMA and every lhsT is a 128-col aligned block
  - everything cast to fp16 (PE streams fp16 at 1 col/cycle, halves DMA;
    PSUM accumulation is fp32)
Device: 12 PE warm-up matmuls bridge the DMA fill (HAM clock gate), then
208 matmuls [K=128, M=128, N=512] (PSUM rows 124-127 hold discarded partials)
+ 13 column-tiled matmul pairs for the last 8 output rows; N packs 2 channels
x 256 output columns (one PSUM bank). DMA issue alternates the two HWDGE
rings ordered by consumption deadline; the last block runs pair-outer so
PSUM drain/stores overlap the final matmuls.
"""
import numpy as np

import concourse.bacc as bacc
import concourse.mybir as mybir
import concourse.tile as tile
from concourse import bass_utils

KS = 13
PAD = 6
S = 4
B, C, H, W = 8, 4, 1024, 1024
OH = OW = 256
NPH = (W + 2 * PAD) // S  # 259
ROWL = C * S * NPH        # 4144
NROW = H + 2 * PAD        # 1036
MDT = mybir.dt.float16
NPDT = np.float16

# row-block grids: t0 tile rows, t1 tile rows (overlapping regrid), sliver rows
ROW_OFFS = [0, 128, 256, 384, 496, 624, 752, 880]
M_TILE = 124

_NC_CACHE = {}


def _host_pack_images(im: np.ndarray) -> np.ndarray:
    """im [8,4,1024,1024] f32 -> [8, 9, 128, ROWL] fp16 row-block tiles."""
    im_pad = np.pad(im, ((0, 0), (0, 0), (PAD, PAD), (PAD, PAD)), mode="edge")
    planes = im_pad.reshape(B, C, NROW, NPH, S).transpose(0, 1, 2, 4, 3)
    rows = (
        np.ascontiguousarray(planes.transpose(0, 2, 1, 3, 4))
        .reshape(B, NROW, ROWL)
        .astype(NPDT)
    )
    img = np.zeros((B, 9, 128, ROWL), NPDT)
    for g, y0 in enumerate(ROW_OFFS):
        img[:, g] = rows[:, y0 : y0 + 128]
    img[:, 8, :41] = rows[:, 992:1033]
    return img


def _host_pack_weights(kernel: np.ndarray) -> np.ndarray:
    """kernel [8,1,13,13] f32 -> [8, 128, 13*256] fp16 banded matrices.

    wfull[b, y, kx*256 + 128 + m] = kernel[b, 0, y - 4m, kx] (zero outside band).
    """
    ker = np.asarray(kernel, np.float32)[:, 0]  # [8,13,13]
    y = np.arange(128)[:, None]
    m = np.arange(256)[None, :] - 128
    ky = y - 4 * m
    valid = (ky >= 0) & (ky < KS)
    kyc = np.clip(ky, 0, KS - 1)
    wk = ker[:, kyc].transpose(0, 3, 1, 2)  # [8, 13(kx), 128(y), 256(m)]
    wfull = np.where(valid[None, None], wk, 0.0)  # [8, kx, y, 256]
    # per-j duplicated layout [8, y, j, kx, 128] so each j group is one
    # contiguous DMA and every lhsT is a 128-col aligned block
    wj = np.zeros((B, 128, 4, KS, 128), np.float32)
    for j in range(4):
        wj[:, :, j] = wfull.transpose(0, 2, 1, 3)[:, :, :, 128 - 32 * j : 256 - 32 * j]
    return np.ascontiguousarray(wj).reshape(B, 128, 4 * KS * 128).astype(NPDT)


def _build_nc():
    nc = bacc.Bacc("TRN2", target_bir_lowering=False, debug=False, num_devices=B)
    img_d = nc.dram_tensor("img", [9, 128, ROWL], MDT, kind="ExternalInput")
    w_d = nc.dram_tensor("wfull", [128, 4 * KS * 128], MDT, kind="ExternalInput")
    out_d = nc.dram_tensor("out", [OH, C * OW], mybir.dt.float32, kind="ExternalOutput")

    with tile.TileContext(nc) as tc:
        with (
            tc.tile_pool(name="wp", bufs=1) as wp,
            tc.tile_pool(name="ip", bufs=1) as ip,
            tc.tile_pool(name="op", bufs=4) as op,
            tc.tile_pool(name="ps", bufs=4, space="PSUM") as ps,
            tc.tile_pool(name="ps1", bufs=1, space="PSUM") as ps1,
        ):
            # weights: per-j slice groups, issued j0, j1 now; j2+j3 go out
            # after img1's second half (j=1's data deadline is tighter than
            # the j2/j3 weight deadlines)
            JG = KS * 128
            wall = wp.tile([128, 4 * KS * 128], MDT, tag="wall")
            nc.scalar.dma_start(wall[:, 0:JG], w_d.ap()[:, 0:JG])
            nc.scalar.dma_start(wall[:, JG : 2 * JG], w_d.ap()[:, JG : 2 * JG])

            # PE warm-up against the HAM clock gate while DMAs land
            warm = wp.tile([128, 512], MDT, tag="warm")
            nc.vector.memset(warm[:].bitcast(mybir.dt.uint16), 0)
            pwarm = ps1.tile([128, 512], mybir.dt.float32, tag="pwarm")
            for wi in range(12):
                nc.tensor.matmul(
                    pwarm[:], warm[:, 0:128], warm[:],
                    start=(wi == 0), stop=(wi == 11), skip_group_check=True,
                )

            imgs = {}
            half = ROWL // 2
            for g in range(9):
                tl = ip.tile([128, ROWL], MDT, tag=f"img{g}")
                eng = nc.sync if g % 2 == 0 else nc.scalar
                if g == 0:
                    # halves so the opening pair-outer MMs start sooner
                    eng.dma_start(tl[:, 0:half], img_d.ap()[g][:, 0:half])
                    eng.dma_start(tl[:, half:], img_d.ap()[g][:, half:])
                elif g == 1:
                    # split across BOTH rings: j=1's deadline is the tightest
                    nc.sync.dma_start(tl[:, 0:half], img_d.ap()[g][:, 0:half])
                    nc.scalar.dma_start(tl[:, half:], img_d.ap()[g][:, half:])
                    nc.scalar.dma_start(wall[:, 2 * JG :], w_d.ap()[:, 2 * JG :])
                elif g == 8:
                    # only 41 rows carry data; don't DMA the zero padding
                    eng.dma_start(tl[0:41, :], img_d.ap()[g][0:41, :])
                else:
                    eng.dma_start(tl[:], img_d.ap()[g])
                imgs[g] = tl

            M = M_TILE
            def do_tile(t):
                psums = []
                for pair in range(2):
                    acc = ps.tile([128, 512], mybir.dt.float32, tag="acc")
                    psums.append(acc)
                n_mm = 4 * KS
                pair_ct = [0, 0]
                for j in range(4):
                    g = 4 * t + j
                    rview = imgs[g][:].rearrange("p (c x) -> p c x", c=C)
                    # first block of the run: pair-outer so the opening 13 MMs
                    # need only wall[:, 0:256] + img0's first channel pair;
                    # last block: pair-outer so pair0's PSUM drains while
                    # pair1's final matmuls still stream
                    if (t == 0 and j == 0) or (t == 1 and j == 3):
                        order = [(kx, pair) for pair in range(2) for kx in range(KS)]
                    else:
                        order = [(kx, pair) for kx in range(KS) for pair in range(2)]
                    for kx, pair in order:
                        u, s = kx // S, kx % S
                        c0 = (j * KS + kx) * 128
                        off = s * NPH + u
                        rhs = rview[:, 2 * pair : 2 * pair + 2, off : off + 256]
                        # full M=128: psum rows M..127 accumulate partial
                        # (wrong) values for the next tile's first oy rows;
                        # they are never copied out. M=128 matmuls measure
                        # ~14 ns faster than M=124.
                        nc.tensor.matmul(
                            psums[pair][:, :], wall[:, c0 : c0 + 128], rhs,
                            start=(pair_ct[pair] == 0),
                            stop=(pair_ct[pair] == n_mm - 1),
                            skip_group_check=True,
                        )
                        pair_ct[pair] += 1
                for pair in range(2):
                    stage = op.tile([128, 512], mybir.dt.float32, tag="stage")
                    oeng = nc.sync if pair == 0 else nc.scalar
                    for h in range(2):
                        nc.vector.tensor_copy(
                            stage[0:M, 256 * h : 256 * h + 256],
                            psums[pair][0:M, 256 * h : 256 * h + 256],
                        )
                        oeng.dma_start(
                            out_d.ap()[
                                M * t : M * t + M,
                                512 * pair + 256 * h : 512 * pair + 256 * h + 256,
                            ],
                            stage[0:M, 256 * h : 256 * h + 256],
                        )

            do_tile(0)
            do_tile(1)
            # sliver: oy 248..255 (8 rows) from rows 992..1032; the two channel
            # pairs run CONCURRENTLY in different PE column groups
            acc2 = ps1.tile([64, 512], mybir.dt.float32, tag="acc2")
            rview = imgs[8][:].rearrange("p (c x) -> p c x", c=C)
            for kx in range(KS):
                u, s = kx // S, kx % S
                c0 = kx * 128
                off = s * NPH + u
                for pair in range(2):
                    rhs = rview[0:41, 2 * pair : 2 * pair + 2, off : off + 256]
                    nc.tensor.matmul(
                        acc2[32 * pair : 32 * pair + 8, :], wall[0:41, c0 : c0 + 8],
                        rhs,
                        start=(kx == 0), stop=(kx == KS - 1),
                        skip_group_check=True,
                        tile_position=(0, 32 * pair),
                    )
            stage2 = op.tile([8, 1024], mybir.dt.float32, tag="stage2")
            for pair in range(2):
                # pipeline: pair0's store drains while pair1's copy runs
                nc.vector.tensor_copy(
                    stage2[:, 512 * pair : 512 * pair + 512],
                    acc2[32 * pair : 32 * pair + 8, :],
                )
                oeng = nc.sync if pair == 0 else nc.scalar
                oeng.dma_start(
                    out_d.ap()[248:256, 512 * pair : 512 * pair + 512],
                    stage2[:, 512 * pair : 512 * pair + 512],
                )

    nc.compile()
    return nc


def get_nc():
    if "nc" not in _NC_CACHE:
        _NC_CACHE["nc"] = _build_nc()
    return _NC_CACHE["nc"]


def kernel(im, kernel, **run_kwargs):
    im = np.asarray(im, np.float32)
    kernel = np.asarray(kernel, np.float32)
    img = _host_pack_images(im)
    wfull = _host_pack_weights(kernel)
    nc = get_nc()
    in_maps = [{"img": img[b], "wfull": wfull[b]} for b in range(B)]
    res = bass_utils.run_bass_kernel_spmd(
        nc, in_maps, core_ids=list(range(B)), **run_kwargs
    )
    out = np.stack([r["out"] for r in res.results])  # [8, 256, 4*256]
    out = np.ascontiguousarray(out.reshape(B, OH, C, OW).transpose(0, 2, 1, 3))
    if run_kwargs:
        return out, res
    return out

